# revision 10
# baseline (speedup 1.0000x reference)
"""Deformable alignment fusion kernel for TRN2, 8-core data-parallel.

Math (per batch b):
  cat    = concat([low, high], ch)                       # (256, H, W)
  offset = conv3x3(cat, w_off) + b_off                   # (18, H, W)  (dy,dx)*9 taps
  aligned= deform_conv(low, offset, w_def) + b_def       # (128, H, W)
  gate   = sigmoid(w_mod @ cat + b_mod)                  # (128, H, W)
  out    = aligned * gate + high

Sharding: core i handles batch b = i//2, rows [64*(i%2), 64*(i%2)+64).

Device algorithm per core (channel-major, fp16 matmul operands):
 - offset conv: 3x3 conv as 18 accumulating matmuls, col-tiled 4x across
   the PE array (chunk j of each group in array cols [32j, 32j+18)), so
   offsets for 4 chunks land PACKED in one [128, 512] PSUM tile.
 - position/index math (clamp, floor-magic, frac, flat index) runs on the
   packed [128, 512] tiles -> full-width DVE instead of 9-partition rows.
 - bilinear sampling in "monomial" form: S = P0 + wx*P1 + wy*P2 + wx*wy*P3
   where P0..P3 are the value / x-diff / y-diff / xy-diff planes of the
   guard-padded low image, gathered at flat index i0 = floor(py)*136 +
   floor(px) from a pixel-major 4-plane table via dma_gather(transpose).
 - the deform conv contraction folds the monomial sum into PSUM:
   psum += w_def_k.T @ P0 ; += w_def_k.T @ (wx*P1) ; ...  (4 matmuls/tap)
 - per-pixel weight broadcast to 128 partitions is split across engines:
   wx via gpsimd.partition_broadcast, wy via a K=1 ones-matmul on the PE
   with ACT (scalar engine) PSUM->SBUF evacuation.
 - gate: two 1x1 matmuls + Sigmoid on the scalar engine.
 - out = (aligned + b_def) [ACT] * gate + high [DVE], fp16 output.
"""

import numpy as np

import concourse.bass as bass
import concourse.tile as tile
from concourse import bacc, mybir
from concourse.bass import ts

dt = mybir.dt
F16 = dt.float16
F32 = dt.float32
I16 = dt.int16
Alu = mybir.AluOpType
Act = mybir.ActivationFunctionType

B, C, H, W = 4, 128, 128, 128
GP = 4                 # guard pad for sampling
HP = H + 2 * GP        # 136
NP = HP * HP           # 18496 padded pixels
HR = 64                # rows per core
NPIX = HR * W          # 8192 pixels per core
NQ = 4                 # quarters (= chunk groups) per core
QP = NPIX // NQ        # 2048 pixels per quarter
QR = QP // W           # 16 rows per quarter
CLAMP_HI = float(H + 2 * GP - 2)  # 134.0 : floor+1 stays inside padded image
PCOL = NPIX // 4       # 2048 columns of the packed [128, PCOL] tiles

# offset channel regrouping: rows 0..8 = dy(tap), rows 9..17 = dx(tap)
PERM = [2 * k for k in range(9)] + [2 * k + 1 for k in range(9)]

# wy broadcast via PE ones-matmul + ACT evac (True) or gpsimd (False)
WY_VIA_PE = True
DEBUG_DUMPS = False


def _ap(t, offset, dims):
    """Raw AP on the same tensor as AP `t`, with explicit [step, count] dims."""
    return bass.AP(tensor=t.tensor, offset=t.offset + offset, ap=list(dims))


def build_program():
    nc = bacc.Bacc("TRN2", debug=False)

    io = {}

    def din(name, shape, d):
        io[name] = nc.dram_tensor(name, shape, d, kind="ExternalInput").ap()
        return io[name]

    din("src4", [NP, 512], F16)           # [pix, (4 planes x 128 ch)]
    din("lowp", [128, 66 * 130], F16)     # rows h0-1..h1+1, W-padded by 1
    din("highp", [128, 66 * 130], F16)
    din("highc", [128, NPIX], F16)        # center high rows, f16
    din("w_off_t", [2, 3, 3, 128, 32], F16)
    din("w_def_t", [9, 128, 128], F16)
    din("w_mod_t", [2, 128, 128], F16)
    din("b_off_p", [128, 1], F32)         # packed: rows 32j+r = b_off_g[r]
    din("b_def_c", [128, 1], F32)
    din("b_mod_c", [128, 1], F32)
    din("base_p", [128, PCOL], F16)       # packed sampling-position bases
    din("ones_c", [1, 128], F16)
    io["idx_scr"] = nc.dram_tensor("idx_scr", [9, NPIX], I16, kind="Internal").ap()
    io["frac_scr"] = nc.dram_tensor("frac_scr", [2, 9, NPIX], F16, kind="Internal").ap()
    io["idx_scr2"] = nc.dram_tensor("idx_scr2", [9, NPIX], I16, kind="Internal").ap()
    out_d = nc.dram_tensor("out", [128, NPIX], F16, kind="ExternalOutput").ap()
    if DEBUG_DUMPS:
        for nm, shp, d in [
            ("dbg_pos", [128, PCOL], F32), ("dbg_posx", [128, PCOL], F32),
            ("dbg_fracx", [128, PCOL], F16), ("dbg_fracy", [128, PCOL], F16),
            ("dbg_idx", [128, PCOL], I16), ("dbg_idxr", [128, 9 * 4 * 128], I16),
            ("dbg_gate", [128, NPIX], F16), ("dbg_g0", [128, 4 * QP], F16),
            ("dbg_wx", [128, QP], F16), ("dbg_wy", [128, QP], F16),
        ]:
            io[nm] = nc.dram_tensor(nm, shp, d, kind="ExternalOutput").ap()

    with tile.TileContext(nc) as tc:
        trace_kernel(tc, io, out_d)

    nc.compile()
    return nc


def trace_kernel(tc, io, out_d):
    nc = tc.nc
    from contextlib import ExitStack

    ctx = ExitStack()
    consts = ctx.enter_context(tc.tile_pool(name="consts", bufs=1))
    npool = ctx.enter_context(tc.tile_pool(name="narrow", bufs=1))
    spool = ctx.enter_context(tc.tile_pool(name="small", bufs=2))
    s1pool = ctx.enter_context(tc.tile_pool(name="small1", bufs=2))
    ppool = ctx.enter_context(tc.tile_pool(name="ps_off", bufs=2, space="PSUM"))
    pgpool = ctx.enter_context(tc.tile_pool(name="ps_gate", bufs=1, space="PSUM"))
    dpool = ctx.enter_context(tc.tile_pool(name="ps_deform", bufs=1, space="PSUM"))
    wypool = ctx.enter_context(tc.tile_pool(name="ps_wy", bufs=1, space="PSUM"))

    # ---------------- images first (critical path), then constants ----------------
    imgpool = ctx.enter_context(tc.tile_pool(name="imgs", bufs=1))
    lowp_sb = imgpool.tile([128, 66, 130], F16)
    highp_sb = imgpool.tile([128, 66, 130], F16)
    # split loads: first 18 rows unblock group 0's offset conv early
    nc.sync.dma_start(
        lowp_sb[:, 0:18, :],
        io["lowp"].rearrange("c (h w) -> c h w", h=66)[:, 0:18, :],
    )
    nc.sync.dma_start(
        highp_sb[:, 0:18, :],
        io["highp"].rearrange("c (h w) -> c h w", h=66)[:, 0:18, :],
    )
    nc.sync.dma_start(
        lowp_sb[:, 18:66, :],
        io["lowp"].rearrange("c (h w) -> c h w", h=66)[:, 18:66, :],
    )
    nc.sync.dma_start(
        highp_sb[:, 18:66, :],
        io["highp"].rearrange("c (h w) -> c h w", h=66)[:, 18:66, :],
    )

    w_off_sb = consts.tile([128, 2, 3, 3, 32], F16)
    nc.sync.dma_start(
        w_off_sb[:], io["w_off_t"].rearrange("cb ky kx c o -> c cb ky kx o")
    )
    w_def_sb = consts.tile([128, 9, 128], F16)
    nc.sync.dma_start(w_def_sb[:], io["w_def_t"].rearrange("k c o -> c k o"))
    w_mod_sb = consts.tile([128, 2, 128], F16)
    nc.sync.dma_start(w_mod_sb[:], io["w_mod_t"].rearrange("cb c o -> c cb o"))
    b_off_sb = consts.tile([128, 1], F32)
    nc.sync.dma_start(b_off_sb[:], io["b_off_p"])
    b_def_sb = consts.tile([128, 1], F32)
    nc.sync.dma_start(b_def_sb[:], io["b_def_c"])
    b_mod_sb = consts.tile([128, 1], F32)
    nc.sync.dma_start(b_mod_sb[:], io["b_mod_c"])
    ones_sb = consts.tile([1, 128], F16)
    nc.sync.dma_start(ones_sb[:], io["ones_c"])
    base_sb = consts.tile([128, PCOL], F16)
    nc.sync.dma_start(base_sb[:], io["base_p"])

    gate_sb = npool.tile([128, NPIX], F16, tag="gate")

    # packed stage-B tiles: partition 32j+k (k<9) = tap k of chunk 4g+j,
    # columns [512g, 512g+512) = group g, natural pixel order per chunk.
    pos = npool.tile([128, PCOL], F32, tag="pos")
    posx = npool.tile([128, PCOL], F32, tag="posx")
    nc.vector.memset(posx[:], 0.0)
    fracy = npool.tile([128, PCOL], F16, tag="fracy")
    fracx = npool.tile([128, PCOL], F16, tag="fracx")
    idx16 = npool.tile([128, PCOL], I16, tag="idx16")
    idxr = npool.tile([128, 9, 4, 128], I16, tag="idxr")

    MAGIC_A = 8388608.0 - 0.5
    MAGIC_B = 8388608.0

    def prange(t_ap, k0, nk, col0, ncol, extra=None):
        """AP over partitions {32j+k0..32j+k0+nk-1, j=0..3} x cols [col0, col0+ncol)."""
        pitch = t_ap.ap[0][0]  # elements per partition row
        dims = [[32 * pitch, 4], [pitch, nk]]
        if extra is not None:
            dims += extra
        else:
            dims += [[1, ncol]]
        return bass.AP(
            tensor=t_ap.tensor, offset=t_ap.offset + k0 * pitch + col0, ap=dims
        )

    # ---------------- stage A+B per group: offset conv -> indices ----------------
    for g in range(NQ):
        sl = ts(g, 512)
        ps = ppool.tile([128, 512], F32, tag="offps")
        n_mm = 0
        for cb in range(2):
            pad = lowp_sb if cb == 0 else highp_sb
            for ky in range(3):
                for kx in range(3):
                    for j in range(4):
                        r0 = (g * 4 + j) * 4
                        nc.tensor.matmul(
                            ps[32 * j : 32 * j + 32, :],
                            lhsT=w_off_sb[:, cb, ky, kx, :],
                            rhs=pad[:, r0 + ky : r0 + ky + 4, kx : kx + 128],
                            start=(n_mm == 0),
                            stop=(n_mm == 17),
                            tile_position=(0, 32 * j),
                            skip_group_check=True,
                        )
                    n_mm += 1
        # evac: pos = psum + b_off + base   (rows 32j+0..17 valid)
        nc.vector.scalar_tensor_tensor(
            out=pos[:, sl], in0=ps[:], scalar=b_off_sb[:], in1=base_sb[:, sl],
            op0=Alu.add, op1=Alu.add,
        )
        if DEBUG_DUMPS:
            nc.sync.dma_start(io["dbg_pos"][:, sl], pos[:, sl])
        # shift x-rows down: posx rows 32j+0..8  <-  pos rows 32j+9..17
        for j in range(4):
            nc.sync.dma_start(
                posx[32 * j : 32 * j + 9, sl],
                pos[32 * j + 9 : 32 * j + 18, sl],
            )
        # clamp in place
        nc.vector.tensor_scalar(
            pos[:, sl], pos[:, sl], 0.0, CLAMP_HI, Alu.max, Alu.min
        )
        nc.vector.tensor_scalar(
            posx[:, sl], posx[:, sl], 0.0, CLAMP_HI, Alu.max, Alu.min
        )
        # frac = pos - floor(pos); floor(v) = ((v - 0.5) + 2^23) - 2^23 for v >= 0
        fty = spool.tile([128, 512], F16, tag="fty")
        nc.vector.tensor_scalar(
            fty[:], pos[:, sl], MAGIC_A, MAGIC_B, Alu.add, Alu.subtract
        )
        nc.vector.tensor_tensor(fracy[:, sl], pos[:, sl], fty[:], Alu.subtract)
        ftx = spool.tile([128, 512], F16, tag="ftx")
        nc.vector.tensor_scalar(
            ftx[:], posx[:, sl], MAGIC_A, MAGIC_B, Alu.add, Alu.subtract
        )
        nc.vector.tensor_tensor(fracx[:, sl], posx[:, sl], ftx[:], Alu.subtract)
        # i0 = floor(py)*HP + floor(px) = (py*HP + px) - (wy*HP + wx), rounded
        nc.vector.scalar_tensor_tensor(
            out=pos[:, sl], in0=pos[:, sl], scalar=float(HP), in1=posx[:, sl],
            op0=Alu.mult, op1=Alu.add,
        )
        nc.vector.scalar_tensor_tensor(
            out=posx[:, sl], in0=fracy[:, sl], scalar=float(HP), in1=fracx[:, sl],
            op0=Alu.mult, op1=Alu.add,
        )
        # (A + 0.25) - B: exact integer +- ~0.04 noise; +0.25 makes the I16
        # cast land correctly under BOTH truncation and round-to-nearest.
        nc.vector.scalar_tensor_tensor(
            out=idx16[:, sl], in0=pos[:, sl], scalar=0.25, in1=posx[:, sl],
            op0=Alu.add, op1=Alu.subtract,
        )
        if DEBUG_DUMPS:
            nc.sync.dma_start(io["dbg_posx"][:, sl], posx[:, sl])
            nc.sync.dma_start(io["dbg_fracx"][:, sl], fracx[:, sl])
            nc.sync.dma_start(io["dbg_fracy"][:, sl], fracy[:, sl])
            nc.sync.dma_start(io["dbg_idx"][:, sl], idx16[:, sl])
        # idx + fracs -> DRAM, natural order: [k, (4g+j)*512 + w]
        for j in range(4):
            doff = (4 * g + j) * 512
            nc.sync.dma_start(
                _ap(io["idx_scr"], doff, [[NPIX, 9], [1, 512]]),
                idx16[32 * j : 32 * j + 9, sl],
            )
            nc.sync.dma_start(
                _ap(io["frac_scr"], doff, [[NPIX, 9], [1, 512]]),
                fracx[32 * j : 32 * j + 9, sl],
            )
            nc.sync.dma_start(
                _ap(io["frac_scr"], 9 * NPIX + doff, [[NPIX, 9], [1, 512]]),
                fracy[32 * j : 32 * j + 9, sl],
            )
        # wrap + replicate: idxr[p16, k, g, s] = idx_scr[k, g*2048 + s*16 + p16]
        for k in range(9):
            off = k * NPIX + g * QP
            A = s1pool.tile([16, 128], I16, tag="idxA")
            nc.sync.dma_start(A[:], _ap(io["idx_scr"], off, [[1, 16], [16, 128]]))
            nc.sync.dma_start(
                _ap(io["idx_scr2"], off, [[128, 16], [1, 128]]), A[:]
            )
            nc.sync.dma_start(
                idxr[:, k, g, :],
                _ap(io["idx_scr2"], off, [[0, 8], [128, 16], [1, 128]]),
            )

    if DEBUG_DUMPS:
        nc.sync.dma_start(
            io["dbg_idxr"].rearrange("p (k q s) -> p k q s", k=9, q=4), idxr[:]
        )
    # ---------------- stage C: gate ----------------
    for ch in range(16):
        r0 = ch * 4
        psg = pgpool.tile([128, 512], F32, tag="gateps")
        for cb in range(2):
            pad = lowp_sb if cb == 0 else highp_sb
            nc.tensor.matmul(
                psg[:],
                lhsT=w_mod_sb[:, cb, :],
                rhs=pad[:, 1 + r0 : 1 + r0 + 4, 1:129],
                start=(cb == 0),
                stop=(cb == 1),
            )
        nc.scalar.activation(
            out=gate_sb[:, ts(ch, 512)], in_=psg[:],
            func=Act.Sigmoid, bias=b_mod_sb[:], scale=1.0,
        )

    if DEBUG_DUMPS:
        nc.sync.dma_start(io["dbg_gate"], gate_sb[:])
    # ---------------- stage D: deformable conv ----------------
    gpool = ctx.enter_context(tc.tile_pool(name="gather", bufs=2))
    wpool = ctx.enter_context(tc.tile_pool(name="wfield", bufs=2))
    tpool = ctx.enter_context(tc.tile_pool(name="tplanes", bufs=2))
    for q in range(NQ):
        dps = dpool.tile([128, QP], F32)  # 4 PSUM banks
        for k in range(9):
            # (1) gather the 4 planes at i0, transposed to channel-major
            G = gpool.tile([128, 4, QP], F16)
            nc.gpsimd.dma_gather(
                out_ap=G[:],
                in_ap=io["src4"],
                idxs_ap=idxr[:, k, q, :],
                num_idxs=QP,
                num_idxs_reg=QP,
                elem_size=512,
                transpose=True,
                single_packet=False,
            )
            # (2) per-pixel weights wx, wy broadcast to 128 partitions
            stgx = s1pool.tile([1, QP], F16, tag="stgx")
            nc.sync.dma_start(
                stgx[:], _ap(io["frac_scr"], k * NPIX + q * QP, [[1, QP]])
            )
            stgy = s1pool.tile([1, QP], F16, tag="stgy")
            nc.sync.dma_start(
                stgy[:],
                _ap(io["frac_scr"], (9 + k) * NPIX + q * QP, [[1, QP]]),
            )
            wx_t = wpool.tile([128, QP], F16, tag="wx")
            nc.gpsimd.partition_broadcast(wx_t[:], stgx[:], channels=128)
            wy_t = wpool.tile([128, QP], F16, tag="wy")
            if WY_VIA_PE:
                for cc in range(4):
                    wyps = wypool.tile([128, 512], F32, tag="wyps")
                    nc.tensor.matmul(
                        wyps[:],
                        lhsT=ones_sb[:],
                        rhs=stgy[:, ts(cc, 512)],
                        start=True,
                        stop=True,
                        tile_position=(0, 0),
                    )
                    nc.scalar.activation(
                        out=wy_t[:, ts(cc, 512)], in_=wyps[:],
                        func=Act.Copy, bias=0.0, scale=1.0,
                    )
            else:
                nc.gpsimd.partition_broadcast(wy_t[:], stgy[:], channels=128)
            if DEBUG_DUMPS and q == 0 and k == 0:
                nc.sync.dma_start(io["dbg_g0"].rearrange("c (p t) -> c p t", p=4), G[:])
                nc.sync.dma_start(io["dbg_wx"], wx_t[:])
                nc.sync.dma_start(io["dbg_wy"], wy_t[:])
            # (3) weighted planes
            T = tpool.tile([128, 3, QP], F16, tag="t")
            nc.vector.tensor_tensor(T[:, 0, :], G[:, 1, :], wx_t[:], Alu.mult)
            nc.vector.tensor_tensor(T[:, 1, :], G[:, 2, :], wy_t[:], Alu.mult)
            nc.vector.tensor_tensor(T[:, 2, :], wx_t[:], wy_t[:], Alu.mult)
            nc.vector.tensor_tensor(T[:, 2, :], G[:, 3, :], T[:, 2, :], Alu.mult)
            # (4) accumulate into the deform psum
            for cc in range(4):
                sl = ts(cc, 512)
                for plane, rhs in enumerate(
                    (G[:, 0, sl], T[:, 0, sl], T[:, 1, sl], T[:, 2, sl])
                ):
                    nc.tensor.matmul(
                        dps[:, sl],
                        lhsT=w_def_sb[:, k, :],
                        rhs=rhs,
                        start=(k == 0 and plane == 0),
                        stop=(k == 8 and plane == 3),
                    )
        # ---------------- stage E: aligned*gate + high ----------------
        for cc in range(4):
            gsl = ts(q * 4 + cc, 512)
            al = spool.tile([128, 512], F16, tag="al")
            nc.scalar.activation(
                out=al[:], in_=dps[:, ts(cc, 512)],
                func=Act.Identity, bias=b_def_sb[:], scale=1.0,
            )
            hc = s1pool.tile([128, 512], F16, tag="hc")
            nc.sync.dma_start(hc[:], io["highc"][:, gsl])
            t1 = spool.tile([128, 512], F16, tag="t1")
            nc.vector.tensor_tensor(t1[:], al[:], gate_sb[:, gsl], Alu.mult)
            nc.vector.tensor_tensor(t1[:], t1[:], hc[:], Alu.add)
            nc.sync.dma_start(out_d[:, gsl], t1[:])

    ctx.close()


# ======================= host side =======================

def _prep_shared(w_off, b_off, w_def, b_def, w_mod, b_mod):
    w_off_g = w_off[PERM]                      # [18, 256, 3, 3]
    w_off_t18 = np.ascontiguousarray(
        w_off_g.reshape(18, 2, 128, 3, 3).transpose(1, 3, 4, 2, 0)
    ).astype(np.float16)                       # [2,3,3,128,18]
    w_off_t = np.zeros((2, 3, 3, 128, 32), np.float16)
    w_off_t[..., :18] = w_off_t18
    b_off_g = b_off[PERM].astype(np.float32)
    b_off_p = np.zeros((128, 1), np.float32)
    for j in range(4):
        b_off_p[32 * j : 32 * j + 18, 0] = b_off_g
    w_def_t = np.ascontiguousarray(
        w_def.reshape(128, 128, 9).transpose(2, 1, 0)
    ).astype(np.float16)                       # [9, c, o]
    w_mod_t = np.ascontiguousarray(
        w_mod.reshape(128, 2, 128).transpose(1, 2, 0)
    ).astype(np.float16)                       # [2, c, o]
    return dict(
        w_off_t=w_off_t,
        b_off_p=b_off_p,
        w_def_t=w_def_t,
        b_def_c=b_def.reshape(128, 1).astype(np.float32),
        w_mod_t=w_mod_t,
        b_mod_c=b_mod.reshape(128, 1).astype(np.float32),
        ones_c=np.ones((1, 128), np.float16),
    )


def _prep_src4(low_b):
    """4-plane pixel-major monomial table of the guard-padded low image."""
    xp = np.zeros((C, HP, HP), np.float32)
    xp[:, GP : GP + H, GP : GP + W] = low_b
    f = xp.reshape(C, NP)
    p0 = f
    p1 = np.zeros_like(f)
    p1[:, :-1] = f[:, 1:] - f[:, :-1]
    p2 = np.zeros_like(f)
    p2[:, :-HP] = f[:, HP:] - f[:, :-HP]
    p3 = np.zeros_like(f)
    p3[:, : -HP - 1] = f[:, HP + 1 :] - f[:, HP:-1] - f[:, 1 : -HP] + f[:, : -HP - 1]
    planes = np.stack([p0, p1, p2, p3], 0)      # [4, C, NP]
    return np.ascontiguousarray(planes.transpose(2, 0, 1)).astype(
        np.float16
    ).reshape(NP, 512)


def _prep_base(h0):
    """Packed sampling-position bases: row 32j+k (k<9) = py base of tap k for
    chunk 4g+j at column [512g + w]; rows 32j+9+k = px base."""
    base = np.zeros((128, PCOL), np.float32)
    w = np.arange(512)
    for g in range(4):
        for j in range(4):
            c = g * 4 + j
            hrow = h0 + c * 4 + w // 128
            wcol = w % 128
            for k in range(9):
                ky, kx = k // 3, k % 3
                base[32 * j + k, g * 512 + w] = hrow + (ky - 1) + GP
                base[32 * j + 9 + k, g * 512 + w] = wcol + (kx - 1) + GP
    return base.astype(np.float16)


def _prep_core(low_b, high_b, h0):
    lp = np.pad(low_b, ((0, 0), (1, 1), (1, 1)))
    hp = np.pad(high_b, ((0, 0), (1, 1), (1, 1)))
    lowp = np.ascontiguousarray(lp[:, h0 : h0 + 66, :]).reshape(128, -1).astype(
        np.float16
    )
    highp = np.ascontiguousarray(hp[:, h0 : h0 + 66, :]).reshape(128, -1).astype(
        np.float16
    )
    highc = np.ascontiguousarray(high_b[:, h0 : h0 + HR, :]).reshape(128, -1).astype(
        np.float16
    )
    return lowp, highp, highc


_PROGRAM_CACHE = {}
_LAST_IN_MAPS = None


def make_in_maps(low_res, high_res, w_off, b_off, w_def, b_def, w_mod, b_mod):
    shared = _prep_shared(
        np.asarray(w_off, np.float32), np.asarray(b_off, np.float32),
        np.asarray(w_def, np.float32), np.asarray(b_def, np.float32),
        np.asarray(w_mod, np.float32), np.asarray(b_mod, np.float32),
    )
    low_res = np.asarray(low_res, np.float32)
    high_res = np.asarray(high_res, np.float32)
    src4_by_batch = [_prep_src4(low_res[b]) for b in range(B)]
    base_by_half = [_prep_base(0), _prep_base(HR)]
    in_maps = []
    for core in range(8):
        b, half = core // 2, core % 2
        h0 = half * HR
        lowp, highp, highc = _prep_core(low_res[b], high_res[b], h0)
        m = dict(shared)
        m["src4"] = src4_by_batch[b]
        m["lowp"] = lowp
        m["highp"] = highp
        m["highc"] = highc
        m["base_p"] = base_by_half[half]
        in_maps.append(m)
    return in_maps


def kernel(low_res, high_res, w_off, b_off, w_def, b_def, w_mod, b_mod):
    global _LAST_IN_MAPS
    if "nc" not in _PROGRAM_CACHE:
        _PROGRAM_CACHE["nc"] = build_program()
    nc = _PROGRAM_CACHE["nc"]

    in_maps = make_in_maps(
        low_res, high_res, w_off, b_off, w_def, b_def, w_mod, b_mod
    )
    _LAST_IN_MAPS = in_maps

    from concourse import bass_utils

    res = bass_utils.run_bass_kernel_spmd(nc, in_maps, core_ids=list(range(8)))
    out = np.empty((B, C, H, W), np.float32)
    for core in range(8):
        b, half = core // 2, core % 2
        out[b, :, half * HR : half * HR + HR, :] = (
            res.results[core]["out"].astype(np.float32).reshape(C, HR, W)
        )
    return out


# revision 15
# speedup vs baseline: 1.2937x; 1.2937x over previous
"""Deformable alignment fusion kernel for TRN2, 8-core data-parallel.

Math (per batch b):
  cat    = concat([low, high], ch)                       # (256, H, W)
  offset = conv3x3(cat, w_off) + b_off                   # (18, H, W)  (dy,dx)*9 taps
  aligned= deform_conv(low, offset, w_def) + b_def       # (128, H, W)
  gate   = sigmoid(w_mod @ cat + b_mod)                  # (128, H, W)
  out    = aligned * gate + high

Sharding: core i handles batch b = i//2, rows [64*(i%2), 64*(i%2)+64).

Device algorithm per core (channel-major, fp16 matmul operands):
 - offset conv: 3x3 conv as 18 accumulating matmuls, col-tiled 4x across
   the PE array (chunk j of each group in array cols [32j, 32j+18)), so
   offsets for 4 chunks land PACKED in one [128, 512] PSUM tile.
 - position/index math (clamp, floor-magic, frac, flat index) runs on the
   packed [128, 512] tiles -> full-width DVE instead of 9-partition rows.
 - bilinear sampling in "monomial" form: S = P0 + wx*P1 + wy*P2 + wx*wy*P3
   where P0..P3 are the value / x-diff / y-diff / xy-diff planes of the
   guard-padded low image, gathered at flat index i0 = floor(py)*136 +
   floor(px) from a pixel-major 4-plane table via dma_gather(transpose).
 - the deform conv contraction folds the monomial sum into PSUM:
   psum += w_def_k.T @ P0 ; += w_def_k.T @ (wx*P1) ; ...  (4 matmuls/tap)
 - per-pixel weight broadcast to 128 partitions is split across engines:
   wx via gpsimd.partition_broadcast, wy via a K=1 ones-matmul on the PE
   with ACT (scalar engine) PSUM->SBUF evacuation.
 - gate: two 1x1 matmuls + Sigmoid on the scalar engine.
 - out = (aligned + b_def) [ACT] * gate + high [DVE], fp16 output.
"""

import numpy as np

import concourse.bass as bass
import concourse.tile as tile
from concourse import bacc, mybir
from concourse.bass import ts

dt = mybir.dt
F16 = dt.float16
F32 = dt.float32
I16 = dt.int16
Alu = mybir.AluOpType
Act = mybir.ActivationFunctionType

B, C, H, W = 4, 128, 128, 128
GP = 4                 # guard pad for sampling
HP = H + 2 * GP        # 136
NP = HP * HP           # 18496 padded pixels
HR = 64                # rows per core
NPIX = HR * W          # 8192 pixels per core
NQ = 4                 # quarters (= chunk groups) per core
QP = NPIX // NQ        # 2048 pixels per quarter
QR = QP // W           # 16 rows per quarter
CLAMP_HI = float(H + 2 * GP - 2)  # 134.0 : floor+1 stays inside padded image
PCOL = NPIX // 4       # 2048 columns of the packed [128, PCOL] tiles

# offset channel regrouping: rows 0..8 = dy(tap), rows 9..17 = dx(tap)
PERM = [2 * k for k in range(9)] + [2 * k + 1 for k in range(9)]

# wy broadcast via PE ones-matmul + ACT evac (True) or gpsimd (False)
WY_VIA_PE = True
DEBUG_DUMPS = False
# ablation for benchmarking: subset of {"gather","seqdma","bcast","tmult","defmm"}
ABLATE = set()


def _ap(t, offset, dims):
    """Raw AP on the same tensor as AP `t`, with explicit [step, count] dims."""
    return bass.AP(tensor=t.tensor, offset=t.offset + offset, ap=list(dims))


def build_program():
    nc = bacc.Bacc("TRN2", debug=False)

    io = {}

    def din(name, shape, d):
        io[name] = nc.dram_tensor(name, shape, d, kind="ExternalInput").ap()
        return io[name]

    din("src4", [NP, 512], F16)           # [pix, (4 planes x 128 ch)]
    din("lowp", [128, 66 * 130], F16)     # rows h0-1..h1+1, W-padded by 1
    din("highp", [128, 66 * 130], F16)
    din("highc", [128, NPIX], F16)        # center high rows, f16
    din("w_off_t", [2, 3, 3, 128, 32], F16)
    din("w_def_t", [9, 128, 128], F16)
    din("w_mod_t", [2, 128, 128], F16)
    din("b_off_p", [128, 1], F32)         # packed: rows 32j+r = b_off_g[r]
    din("b_def_c", [128, 1], F32)
    din("b_mod_c", [128, 1], F32)
    din("base_p", [128, PCOL], F16)       # packed sampling-position bases
    din("ones_c", [1, 128], F16)
    io["idx_scr"] = nc.dram_tensor("idx_scr", [9, NPIX], I16, kind="Internal").ap()
    io["frac_scr"] = nc.dram_tensor("frac_scr", [2, 9, NPIX], F16, kind="Internal").ap()
    io["idx_scr2"] = nc.dram_tensor("idx_scr2", [9, NPIX], I16, kind="Internal").ap()
    out_d = nc.dram_tensor("out", [128, NPIX], F16, kind="ExternalOutput").ap()
    if DEBUG_DUMPS:
        for nm, shp, d in [
            ("dbg_pos", [128, PCOL], F32), ("dbg_posx", [128, PCOL], F32),
            ("dbg_fracx", [128, PCOL], F16), ("dbg_fracy", [128, PCOL], F16),
            ("dbg_idx", [128, PCOL], I16), ("dbg_idxr", [128, 9 * 4 * 128], I16),
            ("dbg_gate", [128, NPIX], F16), ("dbg_g0", [128, 4 * QP], F16),
            ("dbg_wx", [128, QP], F16), ("dbg_wy", [128, QP], F16),
        ]:
            io[nm] = nc.dram_tensor(nm, shp, d, kind="ExternalOutput").ap()

    with tile.TileContext(nc) as tc:
        trace_kernel(tc, io, out_d)

    nc.compile()
    return nc


def trace_kernel(tc, io, out_d):
    nc = tc.nc
    from contextlib import ExitStack

    ctx = ExitStack()
    consts = ctx.enter_context(tc.tile_pool(name="consts", bufs=1))
    npool = ctx.enter_context(tc.tile_pool(name="narrow", bufs=1))
    spool = ctx.enter_context(tc.tile_pool(name="small", bufs=2))
    s1pool = ctx.enter_context(tc.tile_pool(name="small1", bufs=2))
    ppool = ctx.enter_context(tc.tile_pool(name="ps_off", bufs=2, space="PSUM"))
    pgpool = ctx.enter_context(tc.tile_pool(name="ps_gate", bufs=1, space="PSUM"))
    dpool = ctx.enter_context(tc.tile_pool(name="ps_deform", bufs=1, space="PSUM"))
    wypool = ctx.enter_context(tc.tile_pool(name="ps_wy", bufs=1, space="PSUM"))

    # ---------------- images first (critical path), then constants ----------------
    imgpool = ctx.enter_context(tc.tile_pool(name="imgs", bufs=1))
    lowp_sb = imgpool.tile([128, 66, 130], F16)
    highp_sb = imgpool.tile([128, 66, 130], F16)
    # split loads: first 18 rows unblock group 0's offset conv early
    nc.sync.dma_start(
        lowp_sb[:, 0:18, :],
        io["lowp"].rearrange("c (h w) -> c h w", h=66)[:, 0:18, :],
    )
    nc.sync.dma_start(
        highp_sb[:, 0:18, :],
        io["highp"].rearrange("c (h w) -> c h w", h=66)[:, 0:18, :],
    )
    nc.sync.dma_start(
        lowp_sb[:, 18:66, :],
        io["lowp"].rearrange("c (h w) -> c h w", h=66)[:, 18:66, :],
    )
    nc.sync.dma_start(
        highp_sb[:, 18:66, :],
        io["highp"].rearrange("c (h w) -> c h w", h=66)[:, 18:66, :],
    )

    w_off_sb = consts.tile([128, 2, 3, 3, 32], F16)
    nc.sync.dma_start(
        w_off_sb[:], io["w_off_t"].rearrange("cb ky kx c o -> c cb ky kx o")
    )
    w_def_sb = consts.tile([128, 9, 128], F16)
    nc.sync.dma_start(w_def_sb[:], io["w_def_t"].rearrange("k c o -> c k o"))
    w_mod_sb = consts.tile([128, 2, 128], F16)
    nc.sync.dma_start(w_mod_sb[:], io["w_mod_t"].rearrange("cb c o -> c cb o"))
    b_off_sb = consts.tile([128, 1], F32)
    nc.sync.dma_start(b_off_sb[:], io["b_off_p"])
    b_def_sb = consts.tile([128, 1], F32)
    nc.sync.dma_start(b_def_sb[:], io["b_def_c"])
    b_mod_sb = consts.tile([128, 1], F32)
    nc.sync.dma_start(b_mod_sb[:], io["b_mod_c"])
    ones_sb = consts.tile([1, 128], F16)
    nc.sync.dma_start(ones_sb[:], io["ones_c"])
    base_sb = consts.tile([128, PCOL], F16)
    nc.sync.dma_start(base_sb[:], io["base_p"])

    gate_sb = npool.tile([128, NPIX], F16, tag="gate")

    # packed stage-B tiles (per group): partition 32j+k (k<9) = tap k of
    # chunk 4g+j, natural pixel order per chunk.
    bpool = ctx.enter_context(tc.tile_pool(name="bpool", bufs=2))
    idxr = npool.tile([128, 9, 4, 128], I16, tag="idxr")

    MAGIC_A = 8388608.0 - 0.5
    MAGIC_B = 8388608.0

    def prange(t_ap, k0, nk, col0, ncol, extra=None):
        """AP over partitions {32j+k0..32j+k0+nk-1, j=0..3} x cols [col0, col0+ncol)."""
        pitch = t_ap.ap[0][0]  # elements per partition row
        dims = [[32 * pitch, 4], [pitch, nk]]
        if extra is not None:
            dims += extra
        else:
            dims += [[1, ncol]]
        return bass.AP(
            tensor=t_ap.tensor, offset=t_ap.offset + k0 * pitch + col0, ap=dims
        )

    # ---------------- stage A+B per group: offset conv -> indices ----------------
    for g in range(NQ):
        sl = ts(g, 512)
        pos = bpool.tile([128, 512], F32, tag="pos")
        posx = bpool.tile([128, 512], F32, tag="posx")
        nc.vector.memset(posx[:], 0.0)
        fracy = bpool.tile([128, 512], F16, tag="fracy")
        fracx = bpool.tile([128, 512], F16, tag="fracx")
        idx16 = bpool.tile([128, 512], I16, tag="idx16")
        ps = ppool.tile([128, 512], F32, tag="offps")
        n_mm = 0
        for cb in range(2):
            pad = lowp_sb if cb == 0 else highp_sb
            for ky in range(3):
                for kx in range(3):
                    for j in range(4):
                        r0 = (g * 4 + j) * 4
                        nc.tensor.matmul(
                            ps[32 * j : 32 * j + 32, :],
                            lhsT=w_off_sb[:, cb, ky, kx, :],
                            rhs=pad[:, r0 + ky : r0 + ky + 4, kx : kx + 128],
                            start=(n_mm == 0),
                            stop=(n_mm == 17),
                            tile_position=(0, 32 * j),
                            skip_group_check=True,
                        )
                    n_mm += 1
        # evac: pos = psum + b_off + base   (rows 32j+0..17 valid)
        nc.vector.scalar_tensor_tensor(
            out=pos[:], in0=ps[:], scalar=b_off_sb[:], in1=base_sb[:, sl],
            op0=Alu.add, op1=Alu.add,
        )
        if DEBUG_DUMPS:
            nc.sync.dma_start(io["dbg_pos"][:, sl], pos[:])
        # shift x-rows down: posx rows 32j+0..8  <-  pos rows 32j+9..17
        for j in range(4):
            nc.sync.dma_start(
                posx[32 * j : 32 * j + 9, :],
                pos[32 * j + 9 : 32 * j + 18, :],
            )
        # clamp in place
        nc.vector.tensor_scalar(
            pos[:], pos[:], 0.0, CLAMP_HI, Alu.max, Alu.min
        )
        nc.vector.tensor_scalar(
            posx[:], posx[:], 0.0, CLAMP_HI, Alu.max, Alu.min
        )
        # frac = pos - floor(pos); floor(v) = ((v - 0.5) + 2^23) - 2^23 for v >= 0
        fty = spool.tile([128, 512], F16, tag="fty")
        nc.vector.tensor_scalar(
            fty[:], pos[:], MAGIC_A, MAGIC_B, Alu.add, Alu.subtract
        )
        nc.vector.tensor_tensor(fracy[:], pos[:], fty[:], Alu.subtract)
        ftx = spool.tile([128, 512], F16, tag="ftx")
        nc.vector.tensor_scalar(
            ftx[:], posx[:], MAGIC_A, MAGIC_B, Alu.add, Alu.subtract
        )
        nc.vector.tensor_tensor(fracx[:], posx[:], ftx[:], Alu.subtract)
        # i0 = floor(py)*HP + floor(px) = (py*HP + px) - (wy*HP + wx), rounded
        nc.vector.scalar_tensor_tensor(
            out=pos[:], in0=pos[:], scalar=float(HP), in1=posx[:],
            op0=Alu.mult, op1=Alu.add,
        )
        nc.vector.scalar_tensor_tensor(
            out=posx[:], in0=fracy[:], scalar=float(HP), in1=fracx[:],
            op0=Alu.mult, op1=Alu.add,
        )
        # (A + 0.25) - B: exact integer +- ~0.04 noise; +0.25 makes the I16
        # cast land correctly under BOTH truncation and round-to-nearest.
        nc.vector.scalar_tensor_tensor(
            out=idx16[:], in0=pos[:], scalar=0.25, in1=posx[:],
            op0=Alu.add, op1=Alu.subtract,
        )
        if DEBUG_DUMPS:
            nc.sync.dma_start(io["dbg_posx"][:, sl], posx[:])
            nc.sync.dma_start(io["dbg_fracx"][:, sl], fracx[:])
            nc.sync.dma_start(io["dbg_fracy"][:, sl], fracy[:])
            nc.sync.dma_start(io["dbg_idx"][:, sl], idx16[:])
        # idx + fracs -> DRAM, natural order: [k, (4g+j)*512 + w]
        for j in range(4):
            doff = (4 * g + j) * 512
            nc.sync.dma_start(
                _ap(io["idx_scr"], doff, [[NPIX, 9], [1, 512]]),
                idx16[32 * j : 32 * j + 9, :],
            )
            nc.sync.dma_start(
                _ap(io["frac_scr"], doff, [[NPIX, 9], [1, 512]]),
                fracx[32 * j : 32 * j + 9, :],
            )
            nc.sync.dma_start(
                _ap(io["frac_scr"], 9 * NPIX + doff, [[NPIX, 9], [1, 512]]),
                fracy[32 * j : 32 * j + 9, :],
            )
        # wrap + replicate: idxr[p16, k, g, s] = idx_scr[k, g*2048 + s*16 + p16]
        for k in range(9):
            off = k * NPIX + g * QP
            A = s1pool.tile([16, 128], I16, tag="idxA")
            nc.sync.dma_start(A[:], _ap(io["idx_scr"], off, [[1, 16], [16, 128]]))
            nc.scalar.dma_start(
                _ap(io["idx_scr2"], off, [[128, 16], [1, 128]]), A[:]
            )
            nc.sync.dma_start(
                idxr[:, k, g, :],
                _ap(io["idx_scr2"], off, [[0, 8], [128, 16], [1, 128]]),
            )

    if DEBUG_DUMPS:
        nc.sync.dma_start(
            io["dbg_idxr"].rearrange("p (k q s) -> p k q s", k=9, q=4), idxr[:]
        )
    # ---------------- stage C: gate ----------------
    for ch in range(16):
        r0 = ch * 4
        psg = pgpool.tile([128, 512], F32, tag="gateps")
        for cb in range(2):
            pad = lowp_sb if cb == 0 else highp_sb
            nc.tensor.matmul(
                psg[:],
                lhsT=w_mod_sb[:, cb, :],
                rhs=pad[:, 1 + r0 : 1 + r0 + 4, 1:129],
                start=(cb == 0),
                stop=(cb == 1),
            )
        nc.scalar.activation(
            out=gate_sb[:, ts(ch, 512)], in_=psg[:],
            func=Act.Sigmoid, bias=b_mod_sb[:], scale=1.0,
        )

    if DEBUG_DUMPS:
        nc.sync.dma_start(io["dbg_gate"], gate_sb[:])
    # ---------------- stage D: deformable conv ----------------
    gpool = ctx.enter_context(tc.tile_pool(name="gather", bufs=2))
    Gfix = None
    if "gather" in ABLATE and "seqdma" not in ABLATE:
        Gfix = npool.tile([128, 4, QP], F16, tag="Gfix")
        nc.vector.memset(Gfix[:], 0.0)
    wpool = ctx.enter_context(tc.tile_pool(name="wfield", bufs=2))
    tpool = ctx.enter_context(tc.tile_pool(name="tplanes", bufs=2))
    for q in range(NQ):
        dps = None
        if "defmm" not in ABLATE:
            dps = dpool.tile([128, QP], F32)  # 4 PSUM banks
        for k in range(9):
            # (1) gather the 4 planes at i0, transposed to channel-major
            if "gather" in ABLATE and "seqdma" not in ABLATE:
                G = Gfix
            else:
                G = gpool.tile([128, 4, QP], F16)
            if "gather" in ABLATE:
                if "seqdma" in ABLATE:
                    nc.sync.dma_start(
                        G[:],
                        io["src4"].rearrange("p (f c) -> c (p f)", f=4)[
                            :, (k * 1024) % 8192 : (k * 1024) % 8192 + 4 * QP
                        ],
                    )
            else:
                nc.gpsimd.dma_gather(
                    out_ap=G[:],
                    in_ap=io["src4"],
                    idxs_ap=idxr[:, k, q, :],
                    num_idxs=QP,
                    num_idxs_reg=QP,
                    elem_size=512,
                    transpose=True,
                    single_packet=False,
                )
            # (2) per-pixel weights wx, wy broadcast to 128 partitions
            stgx = s1pool.tile([1, QP], F16, tag="stgx")
            nc.scalar.dma_start(
                stgx[:], _ap(io["frac_scr"], k * NPIX + q * QP, [[1, QP]])
            )
            stgy = s1pool.tile([1, QP], F16, tag="stgy")
            nc.scalar.dma_start(
                stgy[:],
                _ap(io["frac_scr"], (9 + k) * NPIX + q * QP, [[1, QP]]),
            )
            wx_t = wy_t = None
            if "bcast" not in ABLATE:
                wx_t = wpool.tile([128, QP], F16, tag="wx")
                nc.gpsimd.partition_broadcast(wx_t[:], stgx[:], channels=128)
                wy_t = wpool.tile([128, QP], F16, tag="wy")
            if "bcast" in ABLATE:
                pass
            elif WY_VIA_PE:
                for cc in range(4):
                    wyps = wypool.tile([128, 512], F32, tag="wyps")
                    nc.tensor.matmul(
                        wyps[:],
                        lhsT=ones_sb[:],
                        rhs=stgy[:, ts(cc, 512)],
                        start=True,
                        stop=True,
                        tile_position=(0, 0),
                    )
                    nc.scalar.activation(
                        out=wy_t[:, ts(cc, 512)], in_=wyps[:],
                        func=Act.Copy, bias=0.0, scale=1.0,
                    )
            else:
                nc.gpsimd.partition_broadcast(wy_t[:], stgy[:], channels=128)
            if DEBUG_DUMPS and q == 0 and k == 0:
                nc.sync.dma_start(io["dbg_g0"].rearrange("c (p t) -> c p t", p=4), G[:])
                nc.sync.dma_start(io["dbg_wx"], wx_t[:])
                nc.sync.dma_start(io["dbg_wy"], wy_t[:])
            # (3) weighted planes
            T = tpool.tile([128, 3, QP], F16, tag="t")
            if "tmult" not in ABLATE and "bcast" not in ABLATE:
                nc.vector.tensor_tensor(T[:, 0, :], G[:, 1, :], wx_t[:], Alu.mult)
                nc.vector.tensor_tensor(T[:, 1, :], G[:, 2, :], wy_t[:], Alu.mult)
                nc.vector.tensor_tensor(T[:, 2, :], wx_t[:], wy_t[:], Alu.mult)
                nc.vector.tensor_tensor(T[:, 2, :], G[:, 3, :], T[:, 2, :], Alu.mult)
            # (4) accumulate into the deform psum
            if "defmm" not in ABLATE:
                use_g = ("tmult" in ABLATE) or ("bcast" in ABLATE)
                for cc in range(4):
                    sl = ts(cc, 512)
                    planes = (
                        (G[:, 0, sl], G[:, 1, sl], G[:, 2, sl], G[:, 3, sl])
                        if use_g
                        else (G[:, 0, sl], T[:, 0, sl], T[:, 1, sl], T[:, 2, sl])
                    )
                    for plane, rhs in enumerate(planes):
                        nc.tensor.matmul(
                            dps[:, sl],
                            lhsT=w_def_sb[:, k, :],
                            rhs=rhs,
                            start=(k == 0 and plane == 0),
                            stop=(k == 8 and plane == 3),
                        )
        # ---------------- stage E: aligned*gate + high ----------------
        for cc in range(4):
            gsl = ts(q * 4 + cc, 512)
            al = spool.tile([128, 512], F16, tag="al")
            if "defmm" in ABLATE:
                nc.vector.memset(al[:], 0.0)
            else:
                nc.scalar.activation(
                    out=al[:], in_=dps[:, ts(cc, 512)],
                    func=Act.Identity, bias=b_def_sb[:], scale=1.0,
                )
            hc = s1pool.tile([128, 512], F16, tag="hc")
            nc.scalar.dma_start(hc[:], io["highc"][:, gsl])
            t1 = spool.tile([128, 512], F16, tag="t1")
            nc.vector.tensor_tensor(t1[:], al[:], gate_sb[:, gsl], Alu.mult)
            nc.vector.tensor_tensor(t1[:], t1[:], hc[:], Alu.add)
            nc.sync.dma_start(out_d[:, gsl], t1[:])

    ctx.close()


# ======================= host side =======================

def _prep_shared(w_off, b_off, w_def, b_def, w_mod, b_mod):
    w_off_g = w_off[PERM]                      # [18, 256, 3, 3]
    w_off_t18 = np.ascontiguousarray(
        w_off_g.reshape(18, 2, 128, 3, 3).transpose(1, 3, 4, 2, 0)
    ).astype(np.float16)                       # [2,3,3,128,18]
    w_off_t = np.zeros((2, 3, 3, 128, 32), np.float16)
    w_off_t[..., :18] = w_off_t18
    b_off_g = b_off[PERM].astype(np.float32)
    b_off_p = np.zeros((128, 1), np.float32)
    for j in range(4):
        b_off_p[32 * j : 32 * j + 18, 0] = b_off_g
    w_def_t = np.ascontiguousarray(
        w_def.reshape(128, 128, 9).transpose(2, 1, 0)
    ).astype(np.float16)                       # [9, c, o]
    w_mod_t = np.ascontiguousarray(
        w_mod.reshape(128, 2, 128).transpose(1, 2, 0)
    ).astype(np.float16)                       # [2, c, o]
    return dict(
        w_off_t=w_off_t,
        b_off_p=b_off_p,
        w_def_t=w_def_t,
        b_def_c=b_def.reshape(128, 1).astype(np.float32),
        w_mod_t=w_mod_t,
        b_mod_c=b_mod.reshape(128, 1).astype(np.float32),
        ones_c=np.ones((1, 128), np.float16),
    )


def _prep_src4(low_b):
    """4-plane pixel-major monomial table of the guard-padded low image."""
    xp = np.zeros((C, HP, HP), np.float32)
    xp[:, GP : GP + H, GP : GP + W] = low_b
    f = xp.reshape(C, NP)
    p0 = f
    p1 = np.zeros_like(f)
    p1[:, :-1] = f[:, 1:] - f[:, :-1]
    p2 = np.zeros_like(f)
    p2[:, :-HP] = f[:, HP:] - f[:, :-HP]
    p3 = np.zeros_like(f)
    p3[:, : -HP - 1] = f[:, HP + 1 :] - f[:, HP:-1] - f[:, 1 : -HP] + f[:, : -HP - 1]
    planes = np.stack([p0, p1, p2, p3], 0)      # [4, C, NP]
    return np.ascontiguousarray(planes.transpose(2, 0, 1)).astype(
        np.float16
    ).reshape(NP, 512)


def _prep_base(h0):
    """Packed sampling-position bases: row 32j+k (k<9) = py base of tap k for
    chunk 4g+j at column [512g + w]; rows 32j+9+k = px base."""
    base = np.zeros((128, PCOL), np.float32)
    w = np.arange(512)
    for g in range(4):
        for j in range(4):
            c = g * 4 + j
            hrow = h0 + c * 4 + w // 128
            wcol = w % 128
            for k in range(9):
                ky, kx = k // 3, k % 3
                base[32 * j + k, g * 512 + w] = hrow + (ky - 1) + GP
                base[32 * j + 9 + k, g * 512 + w] = wcol + (kx - 1) + GP
    return base.astype(np.float16)


def _prep_core(low_b, high_b, h0):
    lp = np.pad(low_b, ((0, 0), (1, 1), (1, 1)))
    hp = np.pad(high_b, ((0, 0), (1, 1), (1, 1)))
    lowp = np.ascontiguousarray(lp[:, h0 : h0 + 66, :]).reshape(128, -1).astype(
        np.float16
    )
    highp = np.ascontiguousarray(hp[:, h0 : h0 + 66, :]).reshape(128, -1).astype(
        np.float16
    )
    highc = np.ascontiguousarray(high_b[:, h0 : h0 + HR, :]).reshape(128, -1).astype(
        np.float16
    )
    return lowp, highp, highc


_PROGRAM_CACHE = {}
_LAST_IN_MAPS = None


def make_in_maps(low_res, high_res, w_off, b_off, w_def, b_def, w_mod, b_mod):
    shared = _prep_shared(
        np.asarray(w_off, np.float32), np.asarray(b_off, np.float32),
        np.asarray(w_def, np.float32), np.asarray(b_def, np.float32),
        np.asarray(w_mod, np.float32), np.asarray(b_mod, np.float32),
    )
    low_res = np.asarray(low_res, np.float32)
    high_res = np.asarray(high_res, np.float32)
    src4_by_batch = [_prep_src4(low_res[b]) for b in range(B)]
    base_by_half = [_prep_base(0), _prep_base(HR)]
    in_maps = []
    for core in range(8):
        b, half = core // 2, core % 2
        h0 = half * HR
        lowp, highp, highc = _prep_core(low_res[b], high_res[b], h0)
        m = dict(shared)
        m["src4"] = src4_by_batch[b]
        m["lowp"] = lowp
        m["highp"] = highp
        m["highc"] = highc
        m["base_p"] = base_by_half[half]
        in_maps.append(m)
    return in_maps


def kernel(low_res, high_res, w_off, b_off, w_def, b_def, w_mod, b_mod):
    global _LAST_IN_MAPS
    if "nc" not in _PROGRAM_CACHE:
        _PROGRAM_CACHE["nc"] = build_program()
    nc = _PROGRAM_CACHE["nc"]

    in_maps = make_in_maps(
        low_res, high_res, w_off, b_off, w_def, b_def, w_mod, b_mod
    )
    _LAST_IN_MAPS = in_maps

    from concourse import bass_utils

    res = bass_utils.run_bass_kernel_spmd(nc, in_maps, core_ids=list(range(8)))
    out = np.empty((B, C, H, W), np.float32)
    for core in range(8):
        b, half = core // 2, core % 2
        out[b, :, half * HR : half * HR + HR, :] = (
            res.results[core]["out"].astype(np.float32).reshape(C, HR, W)
        )
    return out
